# revision 2
# baseline (speedup 1.0000x reference)
"""BilateralSliceApply kernel v3b for 8 Trainium2 NeuronCores.

v3 (PE identity-matmul accumulation for the chain adds) with 512-column
PSUM granularity so both the acc pair and the E pair tiles are 4KB per
partition and can double-buffer inside the 16KB PSUM: full cross-chunk
pipelining. Guide-derived tiles (g16, hinge relus, s16) stay at
1024-column granularity; the chain consumes 512-wide slices.

    acc_pair  = base-matmul  (start)            [PE]
    E_k pair  = plane matmul -> PSUM            [PE]
    pl_k      = evac E_k -> fp16 SBUF           [ACT]
    m_k       = dup(w_k) * pl_k  (fp16 2x)      [DVE]
    acc_pair += Identity @ m_k   (accumulate)   [PE]
    prod      = acc_a * s16   (PSUM read, 1x)   [DVE]
    out       = prod + acc_b  (PSUM read, 1x)   [DVE]

Sharding: 8 shards = batch (4) x H-halves (2), one per core.
"""

import sys

sys.path.insert(0, "/opt/trn_rl_repo")

from contextlib import ExitStack

import numpy as np

import concourse.bacc as bacc
import concourse.bass as bass
import concourse.mybir as mybir
from concourse import tile
from concourse.bass_utils import run_bass_kernel_spmd

N, C, GH, GW, GD = 4, 2, 16, 16, 8
H, W = 2048, 2048
N_CORES = 8
ROWS_PER_CORE = H // 2          # shard = (batch, h-half)
BLK_R = 128                     # rows per block
N_RBLK = ROWS_PER_CORE // BLK_R  # 8
COL_W = 1024                    # columns per guide/x work item
N_CBLK = W // COL_W             # 2
CW = 512                        # columns per PSUM chain chunk
N_PAIRS = 5                     # hinge basis (base, e3, e4, e5, e6)
N_PLANES = 2 * N_PAIRS          # channel-interleaved: p = 2*q + ch

N_RELU_GP = 2                   # hinge relus on gpsimd (rest on DVE)
S16_GP = True                   # x-sum fp16 conversion on gpsimd

_NC_CACHE = {}


def _build_nc(repeat=1):
    key = (CW, repeat, N_RELU_GP, S16_GP)
    if key in _NC_CACHE:
        return _NC_CACHE[key]
    f32 = mybir.dt.float32
    f16 = mybir.dt.float16
    nc = bacc.Bacc("TRN2", target_bir_lowering=False, debug=False,
                   enable_asserts=False, num_devices=N_CORES)
    guide = nc.dram_tensor("guide", [ROWS_PER_CORE, W], f32,
                           kind="ExternalInput").ap()
    xin = nc.dram_tensor("xin", [3, ROWS_PER_CORE, W], f32,
                         kind="ExternalInput").ap()
    tabs = nc.dram_tensor("tabs", [N_RBLK, GW, N_PLANES * BLK_R], f16,
                          kind="ExternalInput").ap()
    rxt = nc.dram_tensor("rxt", [GW, W], f16, kind="ExternalInput").ap()
    ident = nc.dram_tensor("ident", [BLK_R, BLK_R], f16,
                           kind="ExternalInput").ap()
    out = nc.dram_tensor("out", [ROWS_PER_CORE, W], f32,
                         kind="ExternalOutput").ap()

    mult = mybir.AluOpType.mult
    addo = mybir.AluOpType.add
    sub = mybir.AluOpType.subtract
    mx = mybir.AluOpType.max

    with tile.TileContext(nc) as tc:
        with ExitStack() as ctx:
            const_p = ctx.enter_context(tc.tile_pool(name="const", bufs=1))
            tab_p = ctx.enter_context(tc.tile_pool(name="tab", bufs=2))
            g_p = ctx.enter_context(tc.tile_pool(name="g", bufs=3))
            s_p = ctx.enter_context(tc.tile_pool(name="s", bufs=3))
            w_p = ctx.enter_context(tc.tile_pool(name="w", bufs=2))
            pl_p = ctx.enter_context(tc.tile_pool(name="pl", bufs=4))
            ps_p = ctx.enter_context(
                tc.tile_pool(name="ps", bufs=2, space="PSUM"))
            tmp_p = ctx.enter_context(tc.tile_pool(name="tmp", bufs=4))
            out_p = ctx.enter_context(tc.tile_pool(name="o", bufs=4))

            rxt_t = const_p.tile([GW, W], f16)
            nc.sync.dma_start(rxt_t[:], rxt[:])
            id_t = const_p.tile([BLK_R, BLK_R], f16)
            nc.sync.dma_start(id_t[:], ident[:])

            for rb in [r for _ in range(repeat) for r in range(N_RBLK)]:
                tab_t = tab_p.tile([GW, N_PLANES * BLK_R], f16, tag="tab")
                nc.sync.dma_start(tab_t[:], tabs[rb])
                r0 = rb * BLK_R
                for cb in range(N_CBLK):
                    c0 = cb * COL_W
                    g_t = g_p.tile([BLK_R, COL_W], f32, tag="g")
                    nc.sync.dma_start(
                        g_t[:], guide[r0:r0 + BLK_R, c0:c0 + COL_W])
                    s_t = s_p.tile([BLK_R, COL_W], f32, tag="s")
                    nc.gpsimd.dma_start(
                        out=s_t[:], in_=xin[0, r0:r0 + BLK_R, c0:c0 + COL_W])
                    for ch in (1, 2):
                        nc.gpsimd.dma_start(
                            out=s_t[:],
                            in_=xin[ch, r0:r0 + BLK_R, c0:c0 + COL_W],
                            accum_op=addo)

                    g16 = w_p.tile([BLK_R, COL_W], f16, tag="g16")
                    nc.vector.tensor_copy(g16[:], g_t[:])
                    wk = [g16]
                    for i, k in enumerate((4, 5, 6)):
                        relu_eng = nc.gpsimd if i < N_RELU_GP else nc.vector
                        r_t = w_p.tile([BLK_R, COL_W], f16, tag=f"r{k}")
                        relu_eng.tensor_scalar(
                            r_t[:], g16[:], (k - 3.5) / 3.5, 0.0, sub, mx)
                        wk.append(r_t)
                    s16 = s_p.tile([BLK_R, COL_W], f16, tag="s16")
                    (nc.gpsimd if S16_GP else nc.vector).tensor_copy(
                        s16[:], s_t[:])

                    for sc in range(COL_W // CW):
                        u0 = sc * CW
                        cc = c0 + u0

                        def pe_pair(ps_ap, q, start, stop):
                            for half in range(2):
                                p = 2 * q + half
                                nc.tensor.matmul(
                                    ps_ap[:, half, :],
                                    tab_t[:, p * BLK_R:(p + 1) * BLK_R],
                                    rxt_t[:, cc:cc + CW],
                                    start=start, stop=stop)

                        acc_t = ps_p.tile([BLK_R, 2, CW], f32, tag="acc")
                        pe_pair(acc_t[:], 0, start=True, stop=False)

                        def dup(t):
                            return (t[:, u0:u0 + CW].unsqueeze(1)
                                    .broadcast_to([BLK_R, 2, CW]))

                        def id_add(m_t, last):
                            for half in range(2):
                                nc.tensor.matmul(
                                    acc_t[:, half, :], id_t[:],
                                    m_t[:, half, :],
                                    start=False, stop=last)

                        # id-adds deferred one iteration: the in-order PE
                        # starts pair q+1's plane matmul before waiting on
                        # pair q's evac+mult
                        pending = None
                        for i, q in enumerate((1, 2, 3, 4)):
                            ps_t = ps_p.tile([BLK_R, 2, CW], f32, tag="e",
                                             name=f"e{q}")
                            pe_pair(ps_t[:], q, start=True, stop=True)
                            if pending is not None:
                                id_add(pending, last=False)
                            pl_t = pl_p.tile([BLK_R, 2, CW], f16, tag="pl")
                            nc.scalar.copy(pl_t[:], ps_t[:])
                            m_t = tmp_p.tile([BLK_R, 2, CW], f16, tag="m",
                                             name=f"m{q}")
                            nc.vector.tensor_tensor(m_t[:], dup(wk[i]),
                                                    pl_t[:], mult)
                            pending = m_t
                        id_add(pending, last=True)

                        prod = out_p.tile([BLK_R, CW], f16, tag="prod")
                        nc.vector.tensor_tensor(
                            prod[:], acc_t[:, 0], s16[:, u0:u0 + CW], mult)
                        o_t = out_p.tile([BLK_R, CW], f32, tag="o")
                        nc.vector.tensor_tensor(o_t[:], prod[:],
                                                acc_t[:, 1], addo)
                        nc.sync.dma_start(
                            out[r0:r0 + BLK_R, cc:cc + CW], o_t[:])
    nc.compile()
    _NC_CACHE[key] = nc
    return nc


def _build_nc_repeat(repeat):
    return _build_nc(repeat=repeat)


def _host_tables(bilateral_grid):
    """Row tables [N, 2, N_RBLK, GW, N_PLANES*BLK_R] fp16 (plane
    p = 2*q + ch, hinge basis q in (base, e3, e4, e5, e6)) and the
    shared x-interp hat matrix rxt [GW, W] fp16."""
    g64 = np.asarray(bilateral_grid, dtype=np.float64)  # [N,C,GH,GW,GD]
    h = np.arange(H)
    iy = h / (H - 1) * (GH - 1)
    y0 = np.clip(np.floor(iy).astype(np.int64), 0, GH - 1)
    y1 = np.clip(y0 + 1, 0, GH - 1)
    fy = iy - y0
    # grow[n, c, h, j, z]
    grow = ((1.0 - fy)[None, None, :, None, None] * g64[:, :, y0, :, :]
            + fy[None, None, :, None, None] * g64[:, :, y1, :, :])
    D = grow[..., 1:] - grow[..., :-1]
    base = grow[..., 3] + 0.5 * D[..., 3]
    e3 = 3.5 * D[..., 3]
    e4 = 3.5 * (D[..., 4] - D[..., 3])
    e5 = 3.5 * (D[..., 5] - D[..., 4])
    e6 = 3.5 * (D[..., 6] - D[..., 5])
    # [n, c, q, h, j]; plane p = 2*q + c
    pt = np.stack([base, e3, e4, e5, e6], axis=2)
    pt = pt.transpose(0, 2, 1, 3, 4)                 # [n, q, c, h, j]
    pt = pt.reshape(N, N_PLANES, H, GW)
    pt = pt.transpose(0, 2, 3, 1)                    # [n, h, j, p]
    pt = pt.reshape(N, 2, N_RBLK, BLK_R, GW, N_PLANES)
    tabs = pt.transpose(0, 1, 2, 4, 5, 3).reshape(
        N, 2, N_RBLK, GW, N_PLANES * BLK_R).astype(np.float16)

    w = np.arange(W)
    ix = w / (W - 1) * (GW - 1)
    x0 = np.clip(np.floor(ix).astype(np.int64), 0, GW - 1)
    x1 = np.clip(x0 + 1, 0, GW - 1)
    fx = ix - x0
    rxt_f = np.zeros((GW, W))
    rxt_f[x0, w] += 1.0 - fx
    np.add.at(rxt_f, (x1, w), fx)
    rxt = rxt_f.astype(np.float16)
    return tabs, rxt


def kernel(bilateral_grid, guidemap, full_res_input):
    guidemap = np.ascontiguousarray(np.asarray(guidemap), dtype=np.float32)
    full_res_input = np.ascontiguousarray(
        np.asarray(full_res_input), dtype=np.float32)
    tabs, rxt = _host_tables(bilateral_grid)
    ident = np.eye(BLK_R, dtype=np.float16)

    nc = _build_nc()
    in_maps = []
    for core in range(N_CORES):
        n, half = divmod(core, 2)
        r0 = half * ROWS_PER_CORE
        in_maps.append({
            "guide": guidemap[n, r0:r0 + ROWS_PER_CORE],
            "xin": full_res_input[n, :, r0:r0 + ROWS_PER_CORE],
            "tabs": tabs[n, half],
            "rxt": rxt,
            "ident": ident,
        })
    res = run_bass_kernel_spmd(nc, in_maps, list(range(N_CORES)), trace=False)
    out = np.empty((N, 1, H, W), dtype=np.float32)
    for core in range(N_CORES):
        n, half = divmod(core, 2)
        r0 = half * ROWS_PER_CORE
        out[n, 0, r0:r0 + ROWS_PER_CORE] = res.results[core]["out"]
    return out
